# revision 8
# baseline (speedup 1.0000x reference)
"""BalanceL1Loss (hard-negative mining) on 8 Trainium2 NeuronCores.

Data-parallel over batch: each of the 8 cores gets 4 of the 32 images.

Math (matches the torch/jax reference):
    binary        = (gt > 0)
    positive      = binary * mask            -> pos_num = sum(positive)
    negative      = (1 - binary) * mask      -> neg_cnt = sum(negative)
    loss          = |pred - gt|
    pos_loss_sum  = sum(positive * loss)
    negative_num  = min(neg_cnt, 3 * pos_num)
    k             = floor(negative_num)
    neg_loss_sum  = sum of the k largest values of (negative * loss)
    out           = (pos_loss_sum + neg_loss_sum) / (pos_num + negative_num + 1e-6)
    (fallback mean(loss) when pos_num == 0)

Top-k via threshold pivot: with v = negative*loss (zeros included) and any
t >= 0,  f(t) = sum(relu(v - t)) + k*t  is convex, an upper bound on the
top-k sum, and exact at t = (k-th largest of v).  The host picks t from a
1/16 sample of the inputs (numpy), so the device needs a single launch.

The whole numerator folds into two Act-accumulated sums via
    pos_loss + f(t) = ml_sum - sum(min(v,t)) + k*t
                    = ml_sum - (N*t - sum(relu(t - v))) + k*t
so per tile the device does one 16-bit streaming pass:
    Pool: three casting DMAs (f32 DRAM -> f16 SBUF; only gpsimd DMAs can
          cast) — charged at the f16 payload on the DMA engines, which
          halves HBM->SBUF time vs streaming f32
    DVE : diff = pred-gt, nm = (gt<=0)*mask (accum -> nm_sum),
          nv = nm*loss, ml = mask*loss
    Act : loss = |diff| (accum -> loss_sum), relu(t - nv) (accum -> S)
    PE  : ones-matmul column sums of mask and ml -> mask_sum (exact) and
          ml_sum
Counts are exact: nm_sum rides the DVE accumulator of the nm op and
mask_sum comes from f16 {0,1} matmuls accumulated in f32 PSUM, so
pos_num / neg_cnt / k are exact; t only pivots the estimate and
f(t)-topk is quadratically small in the sampling error.  Host combines
per-core partials in float64.

Infra notes: this container's walrus rejects TensorScalarPtr/TensorTensor
on the Pool engine and the abs_max ALU op, and accepts at most one
sem-wait per instruction (see _split_multiwait_bir) — the op placement
above is the optimum under those constraints.
"""

import numpy as np
from contextlib import ExitStack

# ---- problem geometry (hardcoded per contest rules) ----
B, H, W = 32, 768, 768
NCORES = 8
B_LOCAL = B // NCORES              # 4 images per core
P = 128                            # SBUF partitions
N_TOTAL = B * H * W                # 18_874_368
N_LOCAL = B_LOCAL * H * W          # 2_359_296
FREE = N_LOCAL // P                # 18432
WMAX = 3072
MAIN_WIDTHS = [1536, 3072, 3072, 3072, 3072, 3072, 1024, 512]
# columns of each tile's ml product computed on the otherwise-idle Pool
# engine (tensor_tensor mult works there; comparisons don't) to offload
# the DVE, which paces the whole kernel.  512-aligned so each PE chunk
# depends on exactly one writer.
ML_POOL = [1024, 1536, 1536, 1536, 1536, 1536, 0, 0]
# tiles whose ml_sum rides an Act relu-identity accum instead of the PE
# PSUM stream, so the PSUM->SBUF copy + DMA happen before the drain tail
ML_ACT = [False, False, False, False, False, False, True, True]
assert sum(MAIN_WIDTHS) == FREE
NEG_RATIO = 3.0
SSTRIDE = 16                       # host-side sample rate 1/16

_CACHE = {}


def _split_multiwait_bir(bir_bytes):
    """Walrus in this container accepts at most ONE sem-wait per instruction
    (CoreV3GenImpl setupSyncWait: 'Too many sync wait commands'), while
    TileContext packs several.  Hoist all but the last wait of every
    instruction onto fresh same-engine NoOps placed directly before it —
    semantically identical (sem counters are monotone)."""
    import json
    bir = json.loads(bir_bytes)
    n = 0
    for fn in bir["functions"]:
        for blk in fn["blocks"]:
            out = []
            for inst in blk["instructions"]:
                si = inst.get("sync_info")
                ow = (si or {}).get("on_wait") or []
                if len(ow) > 1:
                    for w in ow[:-1]:
                        n += 1
                        out.append({
                            "debug": inst.get("debug"),
                            "engine": inst["engine"],
                            "ins": [],
                            "name": f"I-wsplit{n}",
                            "opcode": "NoOp",
                            "outs": [],
                            "text_hint": "wait_split",
                            "sync_info": {"on_wait": [w], "on_update": []},
                        })
                    si["on_wait"] = [ow[-1]]
                out.append(inst)
            blk["instructions"] = out
    return json.dumps(bir).encode()


def _patch_bass():
    import concourse.bass as bass
    if getattr(bass.Bass, "_wsplit_patched", False):
        return
    orig = bass.Bass.to_json_bytes

    def to_json_bytes(self):
        return _split_multiwait_bir(orig(self))

    bass.Bass.to_json_bytes = to_json_bytes
    bass.Bass._wsplit_patched = True


def _bass_mods():
    import concourse.bass as bass
    import concourse.tile as tile
    from concourse import mybir
    _patch_bass()
    return bass, tile, mybir


def build_main(widths=None):
    """Single fused streaming pass (one launch, nothing O(N) leaves chip).

    inputs : pred, gt, mask  [P, FREE] f32 (DRAM)
             tneg [P, 2] f32 (tneg[:,1] = +t_hat min-trick bias)
    outputs: acc  [P, 4*nt] f32: per-tile accums
                 cols [0,nt)     nm_sum partials (DVE stt accum)
                 cols [nt,2nt)   loss_sum partials (Act abs accum)
                 cols [2nt,3nt)  S partials = sum(relu(t - nv)) (Act)
                 cols [3nt,4nt)  ml_sum partials for ML_ACT tiles (Act)
             sums [2, 512] f32 PE column sums: row0 mask_sum, row1 ml_sum
    """
    bass, tile, mybir = _bass_mods()
    f32, f16 = mybir.dt.float32, mybir.dt.float16
    A = mybir.AluOpType
    AF = mybir.ActivationFunctionType

    if widths is None:
        widths = MAIN_WIDTHS
    nt = len(widths)
    nchunks_mask = sum(w // 512 for w in widths)
    nchunks_ml = sum(w // 512 for w, a in zip(widths, ML_ACT) if not a)

    nc = bass.Bass("TRN2", target_bir_lowering=False, debug=False)
    pred = nc.dram_tensor("pred", [P, FREE], f32, kind="ExternalInput").ap()
    gt = nc.dram_tensor("gt", [P, FREE], f32, kind="ExternalInput").ap()
    mask = nc.dram_tensor("mask", [P, FREE], f32, kind="ExternalInput").ap()
    tneg = nc.dram_tensor("tneg", [P, 2], f32, kind="ExternalInput").ap()
    acc = nc.dram_tensor("acc", [P, 4 * nt], f32, kind="ExternalOutput").ap()
    sums = nc.dram_tensor("sums", [2, 512], f32, kind="ExternalOutput").ap()

    with tile.TileContext(nc) as tc, ExitStack() as ctx:
        io = ctx.enter_context(tc.tile_pool(name="io", bufs=4))
        mid = ctx.enter_context(tc.tile_pool(name="mid", bufs=3))
        st = ctx.enter_context(tc.tile_pool(name="st", bufs=1))
        ps = ctx.enter_context(tc.tile_pool(name="ps", bufs=1, space="PSUM"))

        ones = st.tile([P, 1], f16)
        nc.vector.memset(ones[:], 1.0)
        acc_sb = st.tile([P, 4 * nt], f32)
        nc.vector.memset(acc_sb[:], 0.0)
        ps_mask = ps.tile([1, 512], f32, tag="ps_mask")
        ps_ml = ps.tile([1, 512], f32, tag="ps_ml")
        tn = st.tile([P, 2], f32)

        off = 0
        ci = 0
        cim = 0
        for j, w in enumerate(widths):
            s = bass.ds(off, w)
            # f32 DRAM -> f16 SBUF casting loads (gpsimd/SWDGE is the only
            # path that can cast); charged at f16 payload on the DMA engines.
            tP = io.tile([P, WMAX], f16, tag="tP")
            nc.gpsimd.dma_start(out=tP[:, :w], in_=pred[:, s])
            tG = io.tile([P, WMAX], f16, tag="tG")
            nc.gpsimd.dma_start(out=tG[:, :w], in_=gt[:, s])
            tM = io.tile([P, WMAX], f16, tag="tM")
            nc.gpsimd.dma_start(out=tM[:, :w], in_=mask[:, s])
            if j == 0:
                # small, off the critical DMA-start path
                nc.sync.dma_start(out=tn[:], in_=tneg[:])

            diff = mid.tile([P, WMAX], f16, tag="diff")
            nc.vector.tensor_tensor(diff[:, :w], tP[:, :w], tG[:, :w],
                                    A.subtract)
            lossb = mid.tile([P, WMAX], f16, tag="lossb")
            nc.scalar.activation(lossb[:, :w], diff[:, :w], AF.Abs,
                                 accum_out=acc_sb[:, nt + j:nt + j + 1])
            nmb = mid.tile([P, WMAX], f16, tag="nmb")
            nc.vector.scalar_tensor_tensor(nmb[:, :w], tG[:, :w], 0.0,
                                           tM[:, :w], A.is_le, A.mult,
                                           accum_out=acc_sb[:, j:j + 1])
            ml = mid.tile([P, WMAX], f16, tag="ml")
            wp = ML_POOL[j]
            wd = w - wp
            if wd > 0:
                nc.vector.tensor_tensor(ml[:, :wd], tM[:, :wd],
                                        lossb[:, :wd], A.mult)
            if wp > 0:
                nc.gpsimd.tensor_tensor(ml[:, wd:w], tM[:, wd:w],
                                        lossb[:, wd:w], A.mult)
            nv = mid.tile([P, WMAX], f16, tag="nv")
            nc.vector.tensor_tensor(nv[:, :w], nmb[:, :w], lossb[:, :w],
                                    A.mult)

            # sum(relu(t - nv)) -> S  (min-trick: sum(min(nv,t)) = N*t - S)
            d1 = mid.tile([P, WMAX], f16, tag="d1")
            nc.scalar.activation(d1[:, :w], nv[:, :w], AF.Relu,
                                 scale=-1.0, bias=tn[:, 1:2],
                                 accum_out=acc_sb[:, 2 * nt + j:2 * nt + j + 1])

            # exact mask count: ones-matmul column sums in f32 PSUM
            for c in range(w // 512):
                cs = bass.ds(c * 512, 512)
                nc.tensor.matmul(ps_mask[:], ones[:], tM[:, cs],
                                 start=ci == 0, stop=ci == nchunks_mask - 1)
                ci += 1
            if ML_ACT[j]:
                # tail tiles: ml_sum via Act relu-identity (ml >= 0), so the
                # PSUM->SBUF copy + sums DMA don't sit in the drain tail
                d2 = mid.tile([P, WMAX], f16, tag="d2")
                nc.scalar.activation(d2[:, :w], ml[:, :w], AF.Relu,
                                     accum_out=acc_sb[:, 3 * nt + j:
                                                      3 * nt + j + 1])
            else:
                for c in range(w // 512):
                    cs = bass.ds(c * 512, 512)
                    nc.tensor.matmul(ps_ml[:], ones[:], ml[:, cs],
                                     start=cim == 0, stop=cim == nchunks_ml - 1)
                    cim += 1
            off += w

        row0 = st.tile([1, 512], f32)
        nc.scalar.copy(row0[:], ps_mask[:])
        row1 = st.tile([1, 512], f32)
        nc.scalar.copy(row1[:], ps_ml[:])
        nc.sync.dma_start(out=sums[0:1, :], in_=row0[:])
        nc.sync.dma_start(out=sums[1:2, :], in_=row1[:])
        nc.sync.dma_start(out=acc[:], in_=acc_sb[:])
    return nc


def _get_programs():
    if "main" not in _CACHE:
        _CACHE["main"] = build_main()
    return _CACHE["main"]


def _run_spmd(nc, in_maps, **kw):
    from concourse.bass_utils import run_bass_kernel_spmd
    return run_bass_kernel_spmd(nc, in_maps, list(range(NCORES)), **kw)


def kernel(pred, gt, mask):
    pred = np.ascontiguousarray(np.asarray(pred, dtype=np.float32))
    gt = np.ascontiguousarray(np.asarray(gt, dtype=np.float32))
    mask = np.ascontiguousarray(np.asarray(mask, dtype=np.float32))
    assert pred.shape == (B, H, W), pred.shape

    main = _get_programs()

    # ---- host-side threshold pick from a 1/16 strided sample ----
    ps_ = pred.reshape(-1)[::SSTRIDE]
    gs_ = gt.reshape(-1)[::SSTRIDE]
    ms_ = mask.reshape(-1)[::SSTRIDE]
    vs = np.where(gs_ <= 0.0, ms_, 0.0) * np.abs(ps_ - gs_)
    nm_s = float(np.where(gs_ <= 0.0, ms_, 0.0).sum(dtype=np.float64))
    pm_s = float(np.where(gs_ > 0.0, ms_, 0.0).sum(dtype=np.float64))
    S_ = vs.size
    k_est = min(nm_s, NEG_RATIO * pm_s) * SSTRIDE
    m_rank = int(np.clip(round(k_est / SSTRIDE), 1, S_))
    t_hat = np.float32(max(float(np.partition(vs, S_ - m_rank)[S_ - m_rank]),
                           0.0))

    tneg = np.zeros((P, 2), dtype=np.float32)
    tneg[:, 0] = -t_hat
    tneg[:, 1] = t_hat

    def core_view(x, c):
        return x[c * B_LOCAL:(c + 1) * B_LOCAL].reshape(P, FREE)

    in_maps = [{"pred": core_view(pred, c),
                "gt": core_view(gt, c),
                "mask": core_view(mask, c),
                "tneg": tneg} for c in range(NCORES)]
    res = _run_spmd(main, in_maps).results

    # ---- combine per-core partials (float64) ----
    nt = len(MAIN_WIDTHS)
    nm_sum = 0.0
    loss_sum = 0.0
    S_sum = 0.0
    mask_sum = 0.0
    ml_sum = 0.0
    for c in range(NCORES):
        a = res[c]["acc"].astype(np.float64)
        nm_sum += a[:, 0:nt].sum()
        loss_sum += a[:, nt:2 * nt].sum()
        S_sum += a[:, 2 * nt:3 * nt].sum()
        ml_sum += a[:, 3 * nt:4 * nt].sum()
        r = res[c]["sums"].astype(np.float64)
        mask_sum += r[0].sum()
        ml_sum += r[1].sum()

    pos_num = mask_sum - nm_sum
    neg_cnt = nm_sum

    if pos_num == 0.0:
        return np.asarray(loss_sum / N_TOTAL, dtype=np.float32)

    negative_num = min(neg_cnt, NEG_RATIO * pos_num)
    k = float(np.floor(negative_num))
    t = float(t_hat)

    # pos_loss + neg_loss = ml_sum - (N*t - S) + k*t
    numer = ml_sum - N_TOTAL * t + S_sum + k * t
    balance = numer / (pos_num + negative_num + 1e-6)
    return np.asarray(balance, dtype=np.float32)


# revision 10
# speedup vs baseline: 1.0799x; 1.0799x over previous
"""BalanceL1Loss (hard-negative mining) on 8 Trainium2 NeuronCores.

Data-parallel over batch: each of the 8 cores gets 4 of the 32 images.

Math (matches the torch/jax reference):
    binary        = (gt > 0)
    positive      = binary * mask            -> pos_num = sum(positive)
    negative      = (1 - binary) * mask      -> neg_cnt = sum(negative)
    loss          = |pred - gt|
    pos_loss_sum  = sum(positive * loss)
    negative_num  = min(neg_cnt, 3 * pos_num)
    k             = floor(negative_num)
    neg_loss_sum  = sum of the k largest values of (negative * loss)
    out           = (pos_loss_sum + neg_loss_sum) / (pos_num + negative_num + 1e-6)
    (fallback mean(loss) when pos_num == 0)

Top-k via threshold pivot: with v = negative*loss (zeros included) and any
t >= 0,  f(t) = sum(relu(v - t)) + k*t  is convex, an upper bound on the
top-k sum, and exact at t = (k-th largest of v).  The host picks t from a
1/16 sample of the inputs (numpy), so the device needs a single launch.

The whole numerator folds into two Act-accumulated sums via
    pos_loss + f(t) = ml_sum - sum(min(v,t)) + k*t
                    = ml_sum - (N*t - sum(relu(t - v))) + k*t
so per tile the device does one 16-bit streaming pass:
    Pool: three casting DMAs (f32 DRAM -> f16 SBUF; only gpsimd DMAs can
          cast) — charged at the f16 payload on the DMA engines, which
          halves HBM->SBUF time vs streaming f32
    DVE : diff = pred-gt, nm = (gt<=0)*mask (accum -> nm_sum),
          nv = nm*loss, ml = mask*loss
    Act : loss = |diff| (accum -> loss_sum), relu(t - nv) (accum -> S)
    PE  : ones-matmul column sums of mask and ml -> mask_sum (exact) and
          ml_sum
Counts are exact: nm_sum rides the DVE accumulator of the nm op and
mask_sum comes from f16 {0,1} matmuls accumulated in f32 PSUM, so
pos_num / neg_cnt / k are exact; t only pivots the estimate and
f(t)-topk is quadratically small in the sampling error.  Host combines
per-core partials in float64.

Infra notes: this container's walrus rejects TensorScalarPtr/TensorTensor
on the Pool engine and the abs_max ALU op, and accepts at most one
sem-wait per instruction (see _split_multiwait_bir) — the op placement
above is the optimum under those constraints.
"""

import numpy as np
from contextlib import ExitStack

# ---- problem geometry (hardcoded per contest rules) ----
B, H, W = 32, 768, 768
NCORES = 8
B_LOCAL = B // NCORES              # 4 images per core
P = 128                            # SBUF partitions
N_TOTAL = B * H * W                # 18_874_368
N_LOCAL = B_LOCAL * H * W          # 2_359_296
FREE = N_LOCAL // P                # 18432
WMAX = 3072
MAIN_WIDTHS = [512, 1024, 2048, 3072, 3072, 3072, 3072, 2048, 512]
# columns of each tile's ml product computed on the otherwise-idle Pool
# engine (tensor_tensor mult works there; comparisons don't) to offload
# the DVE, which paces the whole kernel.  512-aligned so each PE chunk
# depends on exactly one writer.  The casting-DMA issues are emitted at
# high priority so Pool ml work can never starve descriptor generation.
ML_POOL = [0, 0, 1024, 1536, 1536, 1536, 1536, 1024, 0]
# tiles whose ml_sum rides an Act relu-identity accum instead of the PE
# PSUM stream, so the PSUM->SBUF copy + DMA happen before the drain tail
ML_ACT = [False, False, False, False, False, False, False, True, True]
assert sum(MAIN_WIDTHS) == FREE
NEG_RATIO = 3.0
SSTRIDE = 16                       # host-side sample rate 1/16

_CACHE = {}


def _split_multiwait_bir(bir_bytes):
    """Walrus in this container accepts at most ONE sem-wait per instruction
    (CoreV3GenImpl setupSyncWait: 'Too many sync wait commands'), while
    TileContext packs several.  Hoist all but the last wait of every
    instruction onto fresh same-engine NoOps placed directly before it —
    semantically identical (sem counters are monotone)."""
    import json
    bir = json.loads(bir_bytes)
    n = 0
    for fn in bir["functions"]:
        for blk in fn["blocks"]:
            out = []
            for inst in blk["instructions"]:
                si = inst.get("sync_info")
                ow = (si or {}).get("on_wait") or []
                if len(ow) > 1:
                    for w in ow[:-1]:
                        n += 1
                        out.append({
                            "debug": inst.get("debug"),
                            "engine": inst["engine"],
                            "ins": [],
                            "name": f"I-wsplit{n}",
                            "opcode": "NoOp",
                            "outs": [],
                            "text_hint": "wait_split",
                            "sync_info": {"on_wait": [w], "on_update": []},
                        })
                    si["on_wait"] = [ow[-1]]
                out.append(inst)
            blk["instructions"] = out
    return json.dumps(bir).encode()


def _patch_bass():
    import concourse.bass as bass
    if getattr(bass.Bass, "_wsplit_patched", False):
        return
    orig = bass.Bass.to_json_bytes

    def to_json_bytes(self):
        return _split_multiwait_bir(orig(self))

    bass.Bass.to_json_bytes = to_json_bytes
    bass.Bass._wsplit_patched = True


def _bass_mods():
    import concourse.bass as bass
    import concourse.tile as tile
    from concourse import mybir
    _patch_bass()
    return bass, tile, mybir


def build_main(widths=None):
    """Single fused streaming pass (one launch, nothing O(N) leaves chip).

    inputs : pred, gt, mask  [P, FREE] f32 (DRAM)
             tneg [P, 2] f32 (tneg[:,1] = +t_hat min-trick bias)
    outputs: acc  [P, 4*nt] f32: per-tile accums
                 cols [0,nt)     nm_sum partials (DVE stt accum)
                 cols [nt,2nt)   loss_sum partials (Act abs accum)
                 cols [2nt,3nt)  S partials = sum(relu(t - nv)) (Act)
                 cols [3nt,4nt)  ml_sum partials for ML_ACT tiles (Act)
             sums [2, 512] f32 PE column sums: row0 mask_sum, row1 ml_sum
    """
    bass, tile, mybir = _bass_mods()
    f32, f16 = mybir.dt.float32, mybir.dt.float16
    A = mybir.AluOpType
    AF = mybir.ActivationFunctionType

    if widths is None:
        widths = MAIN_WIDTHS
    nt = len(widths)
    nchunks_mask = sum(w // 512 for w in widths)
    nchunks_ml = sum(w // 512 for w, a in zip(widths, ML_ACT) if not a)

    nc = bass.Bass("TRN2", target_bir_lowering=False, debug=False)
    pred = nc.dram_tensor("pred", [P, FREE], f32, kind="ExternalInput").ap()
    gt = nc.dram_tensor("gt", [P, FREE], f32, kind="ExternalInput").ap()
    mask = nc.dram_tensor("mask", [P, FREE], f32, kind="ExternalInput").ap()
    tneg = nc.dram_tensor("tneg", [P, 2], f32, kind="ExternalInput").ap()
    acc = nc.dram_tensor("acc", [P, 4 * nt], f32, kind="ExternalOutput").ap()
    sums = nc.dram_tensor("sums", [2, 512], f32, kind="ExternalOutput").ap()

    with tile.TileContext(nc) as tc, ExitStack() as ctx:
        io = ctx.enter_context(tc.tile_pool(name="io", bufs=4))
        mid = ctx.enter_context(tc.tile_pool(name="mid", bufs=3))
        st = ctx.enter_context(tc.tile_pool(name="st", bufs=1))
        ps = ctx.enter_context(tc.tile_pool(name="ps", bufs=1, space="PSUM"))

        ones = st.tile([P, 1], f16)
        nc.vector.memset(ones[:], 1.0)
        acc_sb = st.tile([P, 4 * nt], f32)
        nc.vector.memset(acc_sb[:], 0.0)
        ps_mask = ps.tile([1, 512], f32, tag="ps_mask")
        ps_ml = ps.tile([1, 512], f32, tag="ps_ml")
        tn = st.tile([P, 2], f32)

        off = 0
        ci = 0
        cim = 0
        for j, w in enumerate(widths):
            s = bass.ds(off, w)
            # f32 DRAM -> f16 SBUF casting loads (gpsimd/SWDGE is the only
            # path that can cast); charged at f16 payload on the DMA engines.
            tP = io.tile([P, WMAX], f16, tag="tP")
            tG = io.tile([P, WMAX], f16, tag="tG")
            tM = io.tile([P, WMAX], f16, tag="tM")
            with tc.high_priority(offset=1 << 20):
                nc.gpsimd.dma_start(out=tP[:, :w], in_=pred[:, s])
                nc.gpsimd.dma_start(out=tG[:, :w], in_=gt[:, s])
                nc.gpsimd.dma_start(out=tM[:, :w], in_=mask[:, s])
            if j == 0:
                # small, off the critical DMA-start path
                nc.sync.dma_start(out=tn[:], in_=tneg[:])

            diff = mid.tile([P, WMAX], f16, tag="diff")
            nc.vector.tensor_tensor(diff[:, :w], tP[:, :w], tG[:, :w],
                                    A.subtract)
            lossb = mid.tile([P, WMAX], f16, tag="lossb")
            nc.scalar.activation(lossb[:, :w], diff[:, :w], AF.Abs,
                                 accum_out=acc_sb[:, nt + j:nt + j + 1])
            nmb = mid.tile([P, WMAX], f16, tag="nmb")
            nc.vector.scalar_tensor_tensor(nmb[:, :w], tG[:, :w], 0.0,
                                           tM[:, :w], A.is_le, A.mult,
                                           accum_out=acc_sb[:, j:j + 1])
            ml = mid.tile([P, WMAX], f16, tag="ml")
            wp = ML_POOL[j]
            wd = w - wp
            if wd > 0:
                nc.vector.tensor_tensor(ml[:, :wd], tM[:, :wd],
                                        lossb[:, :wd], A.mult)
            if wp > 0:
                nc.gpsimd.tensor_tensor(ml[:, wd:w], tM[:, wd:w],
                                        lossb[:, wd:w], A.mult)
            nv = mid.tile([P, WMAX], f16, tag="nv")
            nc.vector.tensor_tensor(nv[:, :w], nmb[:, :w], lossb[:, :w],
                                    A.mult)

            # sum(relu(t - nv)) -> S  (min-trick: sum(min(nv,t)) = N*t - S)
            d1 = mid.tile([P, WMAX], f16, tag="d1")
            nc.scalar.activation(d1[:, :w], nv[:, :w], AF.Relu,
                                 scale=-1.0, bias=tn[:, 1:2],
                                 accum_out=acc_sb[:, 2 * nt + j:2 * nt + j + 1])

            # exact mask count: ones-matmul column sums in f32 PSUM
            for c in range(w // 512):
                cs = bass.ds(c * 512, 512)
                nc.tensor.matmul(ps_mask[:], ones[:], tM[:, cs],
                                 start=ci == 0, stop=ci == nchunks_mask - 1)
                ci += 1
            if ML_ACT[j]:
                # tail tiles: ml_sum via Act relu-identity (ml >= 0), so the
                # PSUM->SBUF copy + sums DMA don't sit in the drain tail
                d2 = mid.tile([P, WMAX], f16, tag="d2")
                nc.scalar.activation(d2[:, :w], ml[:, :w], AF.Relu,
                                     accum_out=acc_sb[:, 3 * nt + j:
                                                      3 * nt + j + 1])
            else:
                for c in range(w // 512):
                    cs = bass.ds(c * 512, 512)
                    nc.tensor.matmul(ps_ml[:], ones[:], ml[:, cs],
                                     start=cim == 0, stop=cim == nchunks_ml - 1)
                    cim += 1
            off += w

        row0 = st.tile([1, 512], f32)
        nc.scalar.copy(row0[:], ps_mask[:])
        row1 = st.tile([1, 512], f32)
        nc.scalar.copy(row1[:], ps_ml[:])
        nc.sync.dma_start(out=sums[0:1, :], in_=row0[:])
        nc.sync.dma_start(out=sums[1:2, :], in_=row1[:])
        nc.sync.dma_start(out=acc[:], in_=acc_sb[:])
    return nc


def _get_programs():
    if "main" not in _CACHE:
        _CACHE["main"] = build_main()
    return _CACHE["main"]


def _run_spmd(nc, in_maps, **kw):
    from concourse.bass_utils import run_bass_kernel_spmd
    return run_bass_kernel_spmd(nc, in_maps, list(range(NCORES)), **kw)


def kernel(pred, gt, mask):
    pred = np.ascontiguousarray(np.asarray(pred, dtype=np.float32))
    gt = np.ascontiguousarray(np.asarray(gt, dtype=np.float32))
    mask = np.ascontiguousarray(np.asarray(mask, dtype=np.float32))
    assert pred.shape == (B, H, W), pred.shape

    main = _get_programs()

    # ---- host-side threshold pick from a 1/16 strided sample ----
    ps_ = pred.reshape(-1)[::SSTRIDE]
    gs_ = gt.reshape(-1)[::SSTRIDE]
    ms_ = mask.reshape(-1)[::SSTRIDE]
    vs = np.where(gs_ <= 0.0, ms_, 0.0) * np.abs(ps_ - gs_)
    nm_s = float(np.where(gs_ <= 0.0, ms_, 0.0).sum(dtype=np.float64))
    pm_s = float(np.where(gs_ > 0.0, ms_, 0.0).sum(dtype=np.float64))
    S_ = vs.size
    k_est = min(nm_s, NEG_RATIO * pm_s) * SSTRIDE
    m_rank = int(np.clip(round(k_est / SSTRIDE), 1, S_))
    t_hat = np.float32(max(float(np.partition(vs, S_ - m_rank)[S_ - m_rank]),
                           0.0))

    tneg = np.zeros((P, 2), dtype=np.float32)
    tneg[:, 0] = -t_hat
    tneg[:, 1] = t_hat

    def core_view(x, c):
        return x[c * B_LOCAL:(c + 1) * B_LOCAL].reshape(P, FREE)

    in_maps = [{"pred": core_view(pred, c),
                "gt": core_view(gt, c),
                "mask": core_view(mask, c),
                "tneg": tneg} for c in range(NCORES)]
    res = _run_spmd(main, in_maps).results

    # ---- combine per-core partials (float64) ----
    nt = len(MAIN_WIDTHS)
    nm_sum = 0.0
    loss_sum = 0.0
    S_sum = 0.0
    mask_sum = 0.0
    ml_sum = 0.0
    for c in range(NCORES):
        a = res[c]["acc"].astype(np.float64)
        nm_sum += a[:, 0:nt].sum()
        loss_sum += a[:, nt:2 * nt].sum()
        S_sum += a[:, 2 * nt:3 * nt].sum()
        ml_sum += a[:, 3 * nt:4 * nt].sum()
        r = res[c]["sums"].astype(np.float64)
        mask_sum += r[0].sum()
        ml_sum += r[1].sum()

    pos_num = mask_sum - nm_sum
    neg_cnt = nm_sum

    if pos_num == 0.0:
        return np.asarray(loss_sum / N_TOTAL, dtype=np.float32)

    negative_num = min(neg_cnt, NEG_RATIO * pos_num)
    k = float(np.floor(negative_num))
    t = float(t_hat)

    # pos_loss + neg_loss = ml_sum - (N*t - S) + k*t
    numer = ml_sum - N_TOTAL * t + S_sum + k * t
    balance = numer / (pos_num + negative_num + 1e-6)
    return np.asarray(balance, dtype=np.float32)
